# revision 14
# baseline (speedup 1.0000x reference)
"""Trainium2 Bass kernel for multi-head causal attention (v2).

Problem (hardcoded): x [2, 2048, 1024] fp32, w_qkv [1024, 3072], w_out [1024, 1024].
  qkv = x @ w_qkv; per-head causal softmax attention (16 heads, d=64);
  out = attn_out @ w_out.

Sharding: 8 cores = (2 batches) x (4 head-groups of 4 heads).
v2 changes vs v1:
  - all inputs bf16 (halves DMA + SBUF, FWL on weight loads)
  - S^T matmuls row-paired across the two heads of a pair (concurrent
    64-row tile_position groups)
  - software-pipelined emission: St(g+1) ahead of Sv(g); a filler queue
    interleaves projection chunks + out-proj of the previous q-chunk into
    the attention blocks to plug PE stalls
  - per-(pair, q-chunk) normalization (reciprocal + DMA broadcast) feeding
    a pipelined out-projection, instead of a serial phase-3/4 tail
  - narrowed causal affine_selects (only the invalid diagonal region)
"""
import numpy as np

import concourse.bass as bass
from concourse import bacc
import concourse.mybir as mybir
import concourse.tile as tile

F32 = mybir.dt.float32
F32R = mybir.dt.float32r
BF16 = mybir.dt.bfloat16
AF = mybir.ActivationFunctionType

B, T, C = 2, 2048, 1024
H_TOT, D = 16, 64
HL = 4             # heads per core
DL = HL * D        # 256 local channels
NJ = 4             # q-chunks of 512
NKT = 16           # k-tiles of 128
NCT = 8            # c-tiles of 128 (contraction over C)
SM_SCALE = 1.0 / np.sqrt(D)

_CACHE = {}


def build_nc(reps=1, skip=()):
    nc = bacc.Bacc("TRN2", target_bir_lowering=False)
    xt = nc.dram_tensor("xt", [C, T], BF16, kind="ExternalInput")
    wqk = nc.dram_tensor("wqk", [C, 2 * DL], BF16, kind="ExternalInput")
    wv = nc.dram_tensor("wv", [C, DL], BF16, kind="ExternalInput")
    wo = nc.dram_tensor("wo", [DL, C], F32R, kind="ExternalInput")
    ones_c = nc.dram_tensor("ones_c", [128, 64], F32R, kind="ExternalInput")
    ones_b = nc.dram_tensor("ones_b", [128, 64], BF16, kind="ExternalInput")
    y = nc.dram_tensor("y", [T, C], F32, kind="ExternalOutput")

    with tile.TileContext(nc) as tc:
      for _rep in range(reps):
        with tc.tile_pool(name="persist", bufs=1) as persist, \
             tc.tile_pool(name="dram", bufs=2, space="DRAM") as drampool:
            # ---- persistent SBUF tiles ----
            qk_tiles = [persist.tile([128, T], F32R, tag=f"qk{m}", name=f"qk{m}")
                        for m in range(4)]
            # qk layout: m=0 Q^T pair0, m=1 Q^T pair1, m=2 K^T pair0, m=3 K^T pair1
            v_sb = [persist.tile([128, HL, D + 1], F32R, tag=f"v{t}", name=f"v{t}")
                    for t in range(NKT)]
            at_t = [persist.tile([128, T], F32R, tag=f"at{p}", name=f"at{p}")
                    for p in range(2)]
            ones128 = persist.tile([128, 64], BF16, tag="ones128",
                                   name="ones128")
            xt_sb = [persist.tile([128, T], BF16, tag=f"xt{c}", name=f"xt{c}")
                     for c in range(NCT)]
            wqk_sb = [persist.tile([128, 2 * DL], BF16, tag=f"wqk{c}",
                                   name=f"wqk{c}") for c in range(NCT)]
            wv_sb = [persist.tile([128, DL], BF16, tag=f"wv{c}", name=f"wv{c}")
                     for c in range(NCT)]
            wo_sb = [persist.tile([128, C], F32R, tag=f"wo{i}", name=f"wo{i}")
                     for i in range(2)]

            with tc.tile_pool(name="pexp", bufs=3) as pexp, \
                 tc.tile_pool(name="norm", bufs=2) as normp, \
                 tc.tile_pool(name="ysb", bufs=4) as ysbp, \
                 tc.tile_pool(name="ps2", bufs=2, space="PSUM") as ps2, \
                 tc.tile_pool(name="pot", bufs=1, space="PSUM") as pot, \
                 tc.tile_pool(name="pwork", bufs=1, space="PSUM") as pwork:

                # ---- input DMA: column-chunked xt so chunk-0 work starts early
                nc.sync.dma_start(out=ones128[:], in_=ones_b[:, :])
                for jc in range(NJ):
                    for c in range(NCT):
                        eng = nc.sync
                        eng.dma_start(
                            out=xt_sb[c][:, 512 * jc:512 * (jc + 1)],
                            in_=xt[128 * c:128 * (c + 1),
                                   512 * jc:512 * (jc + 1)])
                    if jc == 0:
                        for c in range(NCT):
                            nc.sync.dma_start(
                                out=wqk_sb[c][:],
                                in_=wqk[128 * c:128 * (c + 1), :])
                        for c in range(NCT):
                            nc.sync.dma_start(
                                out=wv_sb[c][:],
                                in_=wv[128 * c:128 * (c + 1), :])
                        for i in range(2):
                            nc.sync.dma_start(
                                out=wo_sb[i][:],
                                in_=wo[128 * i:128 * (i + 1), :])
                # ones column for the rowsum trick
                for t in range(NKT):
                    nc.sync.dma_start(
                        out=v_sb[t][:, :, D:D + 1],
                        in_=ones_c[:, 0:HL].rearrange("p (h o) -> p h o", o=1))
                if "exp" in skip:
                    p2c = persist.tile([128, 1024], BF16, tag="p2c",
                                       name="p2c")
                    nc.vector.memset(p2c[:], 0.01)
                if "proj" in skip:
                    for m in range(4):
                        nc.vector.memset(qk_tiles[m][:], 0.01)
                    for t in range(NKT):
                        nc.vector.memset(v_sb[t][:, :, 0:D], 0.01)

                # ---- filler queue: small PE/DVE work units interleaved into
                # the attention blocks to plug engine stalls.
                filler = []

                def pump(k):
                    for _ in range(k):
                        if filler:
                            filler.pop(0)()

                def drain():
                    while filler:
                        filler.pop(0)()

                def qk_chunk_steps(m, j):
                    """Emit Q/K projection chunk as filler steps (2 MMs each)."""
                    if "proj" in skip:
                        return []
                    acc = {}

                    def mk(c0):
                        def f():
                            if c0 == 0:
                                acc["t"] = pwork.tile([128, 512], F32,
                                                      tag="acc", name="acc")
                            for c in (c0, c0 + 1):
                                nc.tensor.matmul(
                                    acc["t"][:],
                                    wqk_sb[c][:, 128 * m:128 * (m + 1)],
                                    xt_sb[c][:, 512 * j:512 * (j + 1)],
                                    start=(c == 0), stop=(c == NCT - 1))
                            if c0 == NCT - 2:
                                nc.vector.tensor_copy(
                                    qk_tiles[m][:, 512 * j:512 * (j + 1)],
                                    acc["t"][:])
                        return f
                    return [mk(c0) for c0 in range(0, NCT, 2)]

                def v_tile_steps(t):
                    if "proj" in skip:
                        return []
                    acc = {}

                    def mk(c0):
                        def f():
                            if c0 == 0:
                                acc["t"] = pwork.tile([128, 512], F32,
                                                      tag="acc", name="acc")
                            for c in (c0, c0 + 1):
                                nc.tensor.matmul(
                                    acc["t"][:, 0:DL],
                                    xt_sb[c][:, 128 * t:128 * (t + 1)],
                                    wv_sb[c][:],
                                    start=(c == 0), stop=(c == NCT - 1))
                            if c0 == NCT - 2:
                                nc.vector.tensor_copy(
                                    v_sb[t][:, :, 0:D],
                                    acc["t"][:, 0:DL].rearrange(
                                        "p (h d) -> p h d", h=HL))
                        return f
                    return [mk(c0) for c0 in range(0, NCT, 2)]

                def outproj_steps(j):
                    """Out-projection + DMA for t-tiles of q-chunk j."""
                    if "outproj" in skip:
                        return []
                    steps = []
                    for t in range(4 * j, 4 * j + 4):
                        for oc in range(2):
                            def f(t=t, oc=oc):
                                yps = pwork.tile([128, 512], F32, tag="y",
                                                 name="y")
                                for i in range(2):
                                    nc.tensor.matmul(
                                        yps[:],
                                        at_t[i][:, 128 * t:128 * (t + 1)],
                                        wo_sb[i][:, 512 * oc:512 * (oc + 1)],
                                        start=(i == 0), stop=(i == 1))
                                ysb = ysbp.tile([128, 512], F32, tag="ysb",
                                                name="ysb")
                                nc.vector.tensor_copy(ysb[:], yps[:])
                                nc.sync.dma_start(
                                    out=y[128 * t:128 * (t + 1),
                                          512 * oc:512 * (oc + 1)],
                                    in_=ysb[:])
                            steps.append(f)
                    return steps

                # ---- attention block for (pair, j), St(g+1) emitted ahead of
                # Sv(g); exp on ACT; masks narrowed to the invalid region.
                def st_group(pair, j, g, s2_t):
                    for kk in range(2):        # kk inner, h2 paired adjacent
                        for h2 in range(2):
                            base = 64 * h2
                            kt = 2 * g + kk
                            nc.tensor.matmul(
                                s2_t[h2][:, 512 * kk:512 * (kk + 1)],
                                qk_tiles[2 + pair][base:base + 64,
                                                   128 * kt:128 * (kt + 1)],
                                qk_tiles[pair][base:base + 64,
                                               512 * j:512 * (j + 1)],
                                start=True, stop=True)

                def attn_block(pair, j):
                    nkt = 4 * (j + 1)
                    ktgs = 2 * (j + 1)
                    ot = [pot.tile([65, 512], F32, tag=f"ot{h2}",
                                   name=f"ot{h2}") for h2 in range(2)]
                    s2_cur = [ps2.tile([128, 1024], F32, tag="s2", name="s2")
                              for _ in range(2)]
                    st_group(pair, j, 0, s2_cur)
                    for g in range(ktgs):
                        if "exp" in skip:
                            p2 = [p2c, p2c]
                        else:
                            p2 = [None, None]
                            for h2 in range(2):
                                p2[h2] = pexp.tile([128, 1024], F32R,
                                                   tag="p2", name="p2")
                                nc.scalar.activation(
                                    p2[h2][:], s2_cur[h2][:], AF.Exp,
                                    scale=float(SM_SCALE))
                        pump(2)
                        if g + 1 < ktgs:
                            s2_nxt = [ps2.tile([128, 1024], F32, tag="s2",
                                               name="s2") for _ in range(2)]
                            st_group(pair, j, g + 1, s2_nxt)
                        else:
                            s2_nxt = None
                        if g >= 2 * j and "mask" not in skip \
                                and "exp" not in skip:
                            for h2 in range(2):
                                for kk in range(2):
                                    r = 2 * g + kk - 4 * j
                                    w = 128 * (r + 1)
                                    nc.gpsimd.affine_select(
                                        out=p2[h2][:, 512 * kk:512 * kk + w],
                                        in_=p2[h2][:, 512 * kk:512 * kk + w],
                                        compare_op=mybir.AluOpType.is_ge,
                                        fill=0.0, base=-128 * r,
                                        pattern=[[1, w]],
                                        channel_multiplier=-1)
                        for h2 in range(2):
                            h = 2 * pair + h2
                            for kk in range(2):
                                kt = 2 * g + kk
                                nc.tensor.matmul(
                                    ot[h2][:],
                                    v_sb[kt][:, h, 0:D + 1],
                                    p2[h2][:, 512 * kk:512 * (kk + 1)],
                                    start=(kt == 0), stop=(kt == nkt - 1))
                        pump(2)
                        s2_cur = s2_nxt

                    # ---- per-chunk normalization ----
                    if "norm" in skip:
                        for h2 in range(2):
                            nc.vector.tensor_copy(
                                at_t[pair][64 * h2:64 * h2 + 64,
                                           512 * j:512 * (j + 1)],
                                ot[h2][0:64, :])
                        return
                    rs = normp.tile([1, 1024], F32, tag="rs", name="rs")
                    rsc = normp.tile([1, 1024], F32, tag="rsc", name="rsc")
                    scr = normp.tile([1, 1024], F32, tag="scr", name="scr")
                    rcr = normp.tile([128, 512], BF16, tag="rcr", name="rcr")
                    for h2 in range(2):
                        nc.vector.tensor_copy(
                            rs[0:1, 512 * h2:512 * (h2 + 1)],
                            ot[h2][64:65, :])
                        nc.vector.tensor_copy(
                            at_t[pair][64 * h2:64 * h2 + 64,
                                       512 * j:512 * (j + 1)],
                            ot[h2][0:64, :])
                    nc.vector.reciprocal_approx_accurate(rsc[:], rs[:], scr[:])
                    # stage recip rows at partitions 64 / 96, then broadcast
                    # across partitions with two K=1 matmuls (no DMA latency).
                    nc.vector.tensor_copy(rcr[32:33, :], rsc[0:1, 0:512])
                    nc.vector.tensor_copy(rcr[64:65, :], rsc[0:1, 512:1024])
                    bc = pwork.tile([128, 512], F32, tag="y", name="bc")
                    for h2 in range(2):
                        pb = 32 + 32 * h2
                        nc.tensor.matmul(
                            bc[64 * h2:64 * h2 + 64, :],
                            ones128[pb:pb + 1, 0:64],
                            rcr[pb:pb + 1, :],
                            start=True, stop=True)
                    nc.vector.tensor_mul(
                        at_t[pair][:, 512 * j:512 * (j + 1)],
                        at_t[pair][:, 512 * j:512 * (j + 1)], bc[:])

                # ---- main schedule ----
                for m in range(4):
                    filler.extend(qk_chunk_steps(m, 0))
                for t in range(4):
                    filler.extend(v_tile_steps(t))
                drain()
                for j in range(NJ):
                    # stage filler: projections for j+1, out-proj for j-1
                    if j + 1 < NJ:
                        for m in range(4):
                            filler.extend(qk_chunk_steps(m, j + 1))
                        for t in range(4 * (j + 1), 4 * (j + 1) + 4):
                            filler.extend(v_tile_steps(t))
                    if j >= 1:
                        filler.extend(outproj_steps(j - 1))
                    attn_block(0, j)
                    pump(4)
                    attn_block(1, j)
                    pump(4)
                drain()
                for f in outproj_steps(NJ - 1):
                    f()
    nc.compile()
    return nc


def _get_runner(reps=1):
    """Compile once; return a callable(in_maps) -> list of per-core out dicts."""
    key = ("runner", reps)
    if key in _CACHE:
        return _CACHE[key]
    import jax
    from jax.sharding import Mesh, PartitionSpec
    from jax.experimental.shard_map import shard_map
    from concourse import bass2jax

    nc = build_nc(reps)
    bass2jax.install_neuronx_cc_hook()

    partition_name = (nc.partition_id_tensor.name
                      if nc.partition_id_tensor else None)
    in_names, out_names, out_avals, zero_outs = [], [], [], []
    for alloc in nc.m.functions[0].allocations:
        if not isinstance(alloc, mybir.MemoryLocationSet):
            continue
        name = alloc.memorylocations[0].name
        if alloc.kind == "ExternalInput":
            if name != partition_name:
                in_names.append(name)
        elif alloc.kind == "ExternalOutput":
            out_names.append(name)
            shape = tuple(alloc.tensor_shape)
            dtype = mybir.dt.np(alloc.dtype)
            out_avals.append(jax.core.ShapedArray(shape, dtype))
            zero_outs.append(np.zeros(shape, dtype))
    n_params = len(in_names)
    n_outs = len(out_avals)
    all_in_names = list(in_names) + list(out_names)
    if partition_name is not None:
        all_in_names.append(partition_name)
    donate = tuple(range(n_params, n_params + n_outs))

    def _body(*args):
        operands = list(args)
        if partition_name is not None:
            operands.append(bass2jax.partition_id_tensor())
        outs = bass2jax._bass_exec_p.bind(
            *operands,
            out_avals=tuple(out_avals),
            in_names=tuple(all_in_names),
            out_names=tuple(out_names),
            lowering_input_output_aliases=(),
            sim_require_finite=True,
            sim_require_nnan=True,
            nc=nc,
        )
        return tuple(outs)

    n_cores = 8
    devices = jax.devices()[:n_cores]
    mesh = Mesh(np.asarray(devices), ("core",))
    in_specs = (PartitionSpec("core"),) * (n_params + n_outs)
    out_specs = (PartitionSpec("core"),) * n_outs
    sharded = jax.jit(
        shard_map(_body, mesh=mesh, in_specs=in_specs, out_specs=out_specs,
                  check_rep=False),
        donate_argnums=donate, keep_unused=True)

    def run(in_maps):
        per_core = [[np.asarray(m[name]) for name in in_names] for m in in_maps]
        concat_in = [np.concatenate([per_core[c][i] for c in range(n_cores)],
                                    axis=0) for i in range(n_params)]
        concat_zeros = [np.zeros((n_cores * z.shape[0], *z.shape[1:]), z.dtype)
                        for z in zero_outs]
        out_arrs = sharded(*concat_in, *concat_zeros)
        return [
            {name: np.asarray(out_arrs[i]).reshape(n_cores,
                                                   *out_avals[i].shape)[c]
             for i, name in enumerate(out_names)}
            for c in range(n_cores)
        ]

    _CACHE[key] = run
    return run


def _get_bench(reps=1, skip=()):
    """Zero-transfer bench callable: inputs pre-placed on device, outputs
    left on device (block_until_ready only). No donation."""
    key = ("bench", reps, tuple(skip))
    if key in _CACHE:
        return _CACHE[key]
    import jax
    from jax.sharding import Mesh, PartitionSpec, NamedSharding
    from jax.experimental.shard_map import shard_map
    from concourse import bass2jax

    nc = build_nc(reps, skip)
    bass2jax.install_neuronx_cc_hook()
    partition_name = (nc.partition_id_tensor.name
                      if nc.partition_id_tensor else None)
    in_names, out_names, out_avals, zero_outs = [], [], [], []
    for alloc in nc.m.functions[0].allocations:
        if not isinstance(alloc, mybir.MemoryLocationSet):
            continue
        name = alloc.memorylocations[0].name
        if alloc.kind == "ExternalInput":
            if name != partition_name:
                in_names.append(name)
        elif alloc.kind == "ExternalOutput":
            out_names.append(name)
            shape = tuple(alloc.tensor_shape)
            dtype = mybir.dt.np(alloc.dtype)
            out_avals.append(jax.core.ShapedArray(shape, dtype))
            zero_outs.append(np.zeros(shape, dtype))
    n_params = len(in_names)
    all_in_names = list(in_names) + list(out_names)
    if partition_name is not None:
        all_in_names.append(partition_name)

    def _body(*args):
        operands = list(args)
        if partition_name is not None:
            operands.append(bass2jax.partition_id_tensor())
        outs = bass2jax._bass_exec_p.bind(
            *operands,
            out_avals=tuple(out_avals),
            in_names=tuple(all_in_names),
            out_names=tuple(out_names),
            lowering_input_output_aliases=(),
            sim_require_finite=True,
            sim_require_nnan=True,
            nc=nc,
        )
        return tuple(outs)

    n_cores = 8
    devices = jax.devices()[:n_cores]
    mesh = Mesh(np.asarray(devices), ("core",))
    nouts = len(out_names)
    in_specs = (PartitionSpec("core"),) * (n_params + nouts)
    out_specs = (PartitionSpec("core"),) * nouts
    sharded = jax.jit(
        shard_map(_body, mesh=mesh, in_specs=in_specs, out_specs=out_specs,
                  check_rep=False),
        keep_unused=True)
    shard = NamedSharding(mesh, PartitionSpec("core"))

    def make_args(in_maps):
        per_core = [[np.asarray(m[name]) for name in in_names]
                    for m in in_maps]
        concat_in = [np.concatenate([per_core[c][i] for c in range(n_cores)],
                                    axis=0) for i in range(n_params)]
        concat_zeros = [np.zeros((n_cores * z.shape[0], *z.shape[1:]),
                                 z.dtype) for z in zero_outs]
        return [jax.device_put(a, shard) for a in concat_in + concat_zeros]

    def call(dev_args):
        outs = sharded(*dev_args)
        for o in outs:
            o.block_until_ready()
        return outs

    call.sharded = sharded
    result = (make_args, call)
    _CACHE[key] = result
    return result


def _prep_in_maps(x, w_qkv, w_out):
    import ml_dtypes
    bf16 = ml_dtypes.bfloat16
    x = np.asarray(x, dtype=np.float32)
    w_qkv = np.asarray(w_qkv, dtype=np.float32)
    w_out = np.asarray(w_out, dtype=np.float32)
    in_maps = []
    xts = [np.ascontiguousarray(x[b].T).astype(bf16) for b in range(B)]
    for core in range(8):
        b, g = divmod(core, 4)
        cl, ch = 256 * g, 256 * g + 256
        wqk = np.ascontiguousarray(
            np.concatenate([w_qkv[:, cl:ch], w_qkv[:, C + cl:C + ch]],
                           axis=1)).astype(bf16)
        wv = np.ascontiguousarray(w_qkv[:, 2 * C + cl:2 * C + ch]).astype(bf16)
        wo = np.ascontiguousarray(w_out[cl:ch, :]).astype(np.float32)
        in_maps.append({"xt": xts[b], "wqk": wqk, "wv": wv, "wo": wo,
                        "ones_c": np.ones((128, 64), dtype=np.float32),
                        "ones_b": np.ones((128, 64), dtype=bf16)})
    return in_maps


def kernel(x, w_qkv, w_out):
    run = _get_runner()
    in_maps = _prep_in_maps(x, w_qkv, w_out)
    results = run(in_maps)
    y = np.zeros((B, T, C), dtype=np.float32)
    for core in range(8):
        b = core // 4
        y[b] += results[core]["y"]
    return y


if __name__ == "__main__":
    rng = np.random.default_rng(0)
    x = rng.standard_normal((B, T, C)).astype(np.float32)
    w_qkv = (rng.standard_normal((C, 3 * C)) / np.sqrt(C)).astype(np.float32)
    w_out = (rng.standard_normal((C, C)) / np.sqrt(C)).astype(np.float32)
    y = kernel(x=x, w_qkv=w_qkv, w_out=w_out)
    print("kernel ran, y:", y.shape, y.dtype, float(np.abs(y).max()))


# revision 16
# speedup vs baseline: 1.1140x; 1.1140x over previous
"""Trainium2 Bass kernel for multi-head causal attention (v2).

Problem (hardcoded): x [2, 2048, 1024] fp32, w_qkv [1024, 3072], w_out [1024, 1024].
  qkv = x @ w_qkv; per-head causal softmax attention (16 heads, d=64);
  out = attn_out @ w_out.

Sharding: 8 cores = (2 batches) x (4 head-groups of 4 heads).
v2 changes vs v1:
  - all inputs bf16 (halves DMA + SBUF, FWL on weight loads)
  - S^T matmuls row-paired across the two heads of a pair (concurrent
    64-row tile_position groups)
  - software-pipelined emission: St(g+1) ahead of Sv(g); a filler queue
    interleaves projection chunks + out-proj of the previous q-chunk into
    the attention blocks to plug PE stalls
  - per-(pair, q-chunk) normalization (reciprocal + DMA broadcast) feeding
    a pipelined out-projection, instead of a serial phase-3/4 tail
  - narrowed causal affine_selects (only the invalid diagonal region)
"""
import numpy as np

import concourse.bass as bass
from concourse import bacc
import concourse.mybir as mybir
import concourse.tile as tile

F32 = mybir.dt.float32
F32R = mybir.dt.float32r
BF16 = mybir.dt.bfloat16
AF = mybir.ActivationFunctionType

B, T, C = 2, 2048, 1024
H_TOT, D = 16, 64
HL = 4             # heads per core
DL = HL * D        # 256 local channels
NJ = 4             # q-chunks of 512
NKT = 16           # k-tiles of 128
NCT = 8            # c-tiles of 128 (contraction over C)
SM_SCALE = 1.0 / np.sqrt(D)

_CACHE = {}


def build_nc(reps=1, skip=()):
    nc = bacc.Bacc("TRN2", target_bir_lowering=False)
    xt = nc.dram_tensor("xt", [C, T], BF16, kind="ExternalInput")
    wqk = nc.dram_tensor("wqk", [C, 2 * DL], BF16, kind="ExternalInput")
    wv = nc.dram_tensor("wv", [C, DL], BF16, kind="ExternalInput")
    wo = nc.dram_tensor("wo", [DL, C], F32R, kind="ExternalInput")
    ones_c = nc.dram_tensor("ones_c", [128, 64], F32R, kind="ExternalInput")
    ones_b = nc.dram_tensor("ones_b", [128, 64], BF16, kind="ExternalInput")
    y = nc.dram_tensor("y", [T, C], F32, kind="ExternalOutput")

    with tile.TileContext(nc) as tc:
      for _rep in range(reps):
        with tc.tile_pool(name="persist", bufs=1) as persist, \
             tc.tile_pool(name="dram", bufs=2, space="DRAM") as drampool:
            # ---- persistent SBUF tiles ----
            qk_tiles = [persist.tile([128, T], BF16, tag=f"qk{m}", name=f"qk{m}")
                        for m in range(4)]
            # qk layout: m=0 Q^T pair0, m=1 Q^T pair1, m=2 K^T pair0, m=3 K^T pair1
            v_sb = [persist.tile([128, HL, D + 1], F32R, tag=f"v{t}", name=f"v{t}")
                    for t in range(NKT)]
            at_t = [persist.tile([128, T], F32R, tag=f"at{p}", name=f"at{p}")
                    for p in range(2)]
            ones128 = persist.tile([128, 64], BF16, tag="ones128",
                                   name="ones128")
            xt_sb = [persist.tile([128, T], BF16, tag=f"xt{c}", name=f"xt{c}")
                     for c in range(NCT)]
            wqk_sb = [persist.tile([128, 2 * DL], BF16, tag=f"wqk{c}",
                                   name=f"wqk{c}") for c in range(NCT)]
            wv_sb = [persist.tile([128, DL], BF16, tag=f"wv{c}", name=f"wv{c}")
                     for c in range(NCT)]
            wo_sb = [persist.tile([128, C], F32R, tag=f"wo{i}", name=f"wo{i}")
                     for i in range(2)]

            with tc.tile_pool(name="pexp", bufs=3) as pexp, \
                 tc.tile_pool(name="norm", bufs=2) as normp, \
                 tc.tile_pool(name="ysb", bufs=4) as ysbp, \
                 tc.tile_pool(name="ps2", bufs=2, space="PSUM") as ps2, \
                 tc.tile_pool(name="pot", bufs=1, space="PSUM") as pot, \
                 tc.tile_pool(name="pwork", bufs=1, space="PSUM") as pwork:

                # ---- input DMA: column-chunked xt so chunk-0 work starts early
                nc.sync.dma_start(out=ones128[:], in_=ones_b[:, :])
                for jc in range(NJ):
                    for c in range(NCT):
                        eng = nc.sync if c % 2 == 0 else nc.scalar
                        eng.dma_start(
                            out=xt_sb[c][:, 512 * jc:512 * (jc + 1)],
                            in_=xt[128 * c:128 * (c + 1),
                                   512 * jc:512 * (jc + 1)])
                    if jc == 0:
                        for c in range(NCT):
                            nc.sync.dma_start(
                                out=wqk_sb[c][:],
                                in_=wqk[128 * c:128 * (c + 1), :])
                        for c in range(NCT):
                            nc.scalar.dma_start(
                                out=wv_sb[c][:],
                                in_=wv[128 * c:128 * (c + 1), :])
                        for i in range(2):
                            nc.scalar.dma_start(
                                out=wo_sb[i][:],
                                in_=wo[128 * i:128 * (i + 1), :])
                # ones column for the rowsum trick
                for t in range(NKT):
                    nc.scalar.dma_start(
                        out=v_sb[t][:, :, D:D + 1],
                        in_=ones_c[:, 0:HL].rearrange("p (h o) -> p h o", o=1))
                if "exp" in skip:
                    p2c = persist.tile([128, 1024], BF16, tag="p2c",
                                       name="p2c")
                    nc.vector.memset(p2c[:], 0.01)
                if "proj" in skip:
                    for m in range(4):
                        nc.vector.memset(qk_tiles[m][:], 0.01)
                    for t in range(NKT):
                        nc.vector.memset(v_sb[t][:, :, 0:D], 0.01)

                # ---- filler queue: small PE/DVE work units interleaved into
                # the attention blocks to plug engine stalls.
                filler = []

                def pump(k):
                    for _ in range(k):
                        if filler:
                            filler.pop(0)()

                def drain():
                    while filler:
                        filler.pop(0)()

                def qk_chunk_steps(m, j):
                    """Emit Q/K projection chunk as filler steps (2 MMs each)."""
                    if "proj" in skip:
                        return []
                    acc = {}

                    def mk(c0):
                        def f():
                            if c0 == 0:
                                acc["t"] = pwork.tile([128, 512], F32,
                                                      tag="acc", name="acc")
                            for c in (c0, c0 + 1):
                                nc.tensor.matmul(
                                    acc["t"][:],
                                    wqk_sb[c][:, 128 * m:128 * (m + 1)],
                                    xt_sb[c][:, 512 * j:512 * (j + 1)],
                                    start=(c == 0), stop=(c == NCT - 1))
                            if c0 == NCT - 2:
                                nc.vector.tensor_copy(
                                    qk_tiles[m][:, 512 * j:512 * (j + 1)],
                                    acc["t"][:])
                        return f
                    return [mk(c0) for c0 in range(0, NCT, 2)]

                def v_tile_steps(t):
                    if "proj" in skip:
                        return []
                    acc = {}

                    def mk(c0):
                        def f():
                            if c0 == 0:
                                acc["t"] = pwork.tile([128, 512], F32,
                                                      tag="acc", name="acc")
                            for c in (c0, c0 + 1):
                                nc.tensor.matmul(
                                    acc["t"][:, 0:DL],
                                    xt_sb[c][:, 128 * t:128 * (t + 1)],
                                    wv_sb[c][:],
                                    start=(c == 0), stop=(c == NCT - 1))
                            if c0 == NCT - 2:
                                nc.vector.tensor_copy(
                                    v_sb[t][:, :, 0:D],
                                    acc["t"][:, 0:DL].rearrange(
                                        "p (h d) -> p h d", h=HL))
                        return f
                    return [mk(c0) for c0 in range(0, NCT, 2)]

                def outproj_steps(j):
                    """Out-projection + DMA for t-tiles of q-chunk j."""
                    if "outproj" in skip:
                        return []
                    steps = []
                    for t in range(4 * j, 4 * j + 4):
                        for oc in range(2):
                            def f(t=t, oc=oc):
                                yps = pwork.tile([128, 512], F32, tag="y",
                                                 name="y")
                                for i in range(2):
                                    nc.tensor.matmul(
                                        yps[:],
                                        at_t[i][:, 128 * t:128 * (t + 1)],
                                        wo_sb[i][:, 512 * oc:512 * (oc + 1)],
                                        start=(i == 0), stop=(i == 1))
                                ysb = ysbp.tile([128, 512], F32, tag="ysb",
                                                name="ysb")
                                nc.vector.tensor_copy(ysb[:], yps[:])
                                nc.sync.dma_start(
                                    out=y[128 * t:128 * (t + 1),
                                          512 * oc:512 * (oc + 1)],
                                    in_=ysb[:])
                            steps.append(f)
                    return steps

                # ---- attention block for (pair, j), St(g+1) emitted ahead of
                # Sv(g); exp on ACT; masks narrowed to the invalid region.
                def st_group(pair, j, g, s2_t):
                    for kk in range(2):        # kk inner, h2 paired adjacent
                        for h2 in range(2):
                            base = 64 * h2
                            kt = 2 * g + kk
                            nc.tensor.matmul(
                                s2_t[h2][:, 512 * kk:512 * (kk + 1)],
                                qk_tiles[2 + pair][base:base + 64,
                                                   128 * kt:128 * (kt + 1)],
                                qk_tiles[pair][base:base + 64,
                                               512 * j:512 * (j + 1)],
                                start=True, stop=True)

                def attn_block(pair, j):
                    nkt = 4 * (j + 1)
                    ktgs = 2 * (j + 1)
                    ot = [pot.tile([65, 512], F32, tag=f"ot{h2}",
                                   name=f"ot{h2}") for h2 in range(2)]
                    s2_cur = [ps2.tile([128, 1024], F32, tag="s2", name="s2")
                              for _ in range(2)]
                    st_group(pair, j, 0, s2_cur)
                    for g in range(ktgs):
                        if "exp" in skip:
                            p2 = [p2c, p2c]
                        else:
                            p2 = [None, None]
                            for h2 in range(2):
                                p2[h2] = pexp.tile([128, 1024], F32R,
                                                   tag="p2", name="p2")
                                nc.scalar.activation(
                                    p2[h2][:], s2_cur[h2][:], AF.Exp,
                                    scale=float(SM_SCALE))
                        pump(2)
                        if g + 1 < ktgs:
                            s2_nxt = [ps2.tile([128, 1024], F32, tag="s2",
                                               name="s2") for _ in range(2)]
                            st_group(pair, j, g + 1, s2_nxt)
                        else:
                            s2_nxt = None
                        if g >= 2 * j and "mask" not in skip \
                                and "exp" not in skip:
                            for h2 in range(2):
                                for kk in range(2):
                                    r = 2 * g + kk - 4 * j
                                    w = 128 * (r + 1)
                                    nc.gpsimd.affine_select(
                                        out=p2[h2][:, 512 * kk:512 * kk + w],
                                        in_=p2[h2][:, 512 * kk:512 * kk + w],
                                        compare_op=mybir.AluOpType.is_ge,
                                        fill=0.0, base=-128 * r,
                                        pattern=[[1, w]],
                                        channel_multiplier=-1)
                        for h2 in range(2):
                            h = 2 * pair + h2
                            for kk in range(2):
                                kt = 2 * g + kk
                                nc.tensor.matmul(
                                    ot[h2][:],
                                    v_sb[kt][:, h, 0:D + 1],
                                    p2[h2][:, 512 * kk:512 * (kk + 1)],
                                    start=(kt == 0), stop=(kt == nkt - 1))
                        pump(2)
                        s2_cur = s2_nxt

                    # ---- per-chunk normalization ----
                    if "norm" in skip:
                        for h2 in range(2):
                            nc.vector.tensor_copy(
                                at_t[pair][64 * h2:64 * h2 + 64,
                                           512 * j:512 * (j + 1)],
                                ot[h2][0:64, :])
                        return
                    rs = normp.tile([1, 1024], F32, tag="rs", name="rs")
                    rsc = normp.tile([1, 1024], F32, tag="rsc", name="rsc")
                    scr = normp.tile([1, 1024], F32, tag="scr", name="scr")
                    rcr = normp.tile([128, 512], BF16, tag="rcr", name="rcr")
                    for h2 in range(2):
                        nc.vector.tensor_copy(
                            rs[0:1, 512 * h2:512 * (h2 + 1)],
                            ot[h2][64:65, :])
                        nc.vector.tensor_copy(
                            at_t[pair][64 * h2:64 * h2 + 64,
                                       512 * j:512 * (j + 1)],
                            ot[h2][0:64, :])
                    nc.vector.reciprocal_approx_accurate(rsc[:], rs[:], scr[:])
                    # stage recip rows at partitions 32 / 64, then broadcast
                    # across partitions with two K=1 matmuls (no DMA latency).
                    nc.vector.tensor_copy(rcr[32:33, :], rsc[0:1, 0:512])
                    nc.vector.tensor_copy(rcr[64:65, :], rsc[0:1, 512:1024])

                    def finish_norm(pair=pair, j=j, rcr=rcr):
                        bc = pwork.tile([128, 512], F32, tag="y", name="bc")
                        for h2 in range(2):
                            pb = 32 + 32 * h2
                            nc.tensor.matmul(
                                bc[64 * h2:64 * h2 + 64, :],
                                ones128[pb:pb + 1, 0:64],
                                rcr[pb:pb + 1, :],
                                start=True, stop=True)
                        nc.vector.tensor_mul(
                            at_t[pair][:, 512 * j:512 * (j + 1)],
                            at_t[pair][:, 512 * j:512 * (j + 1)], bc[:])
                    filler.append(finish_norm)

                # ---- main schedule ----
                for m in range(4):
                    filler.extend(qk_chunk_steps(m, 0))
                for t in range(4):
                    filler.extend(v_tile_steps(t))
                drain()
                for j in range(NJ):
                    # stage filler: projections for j+1, out-proj for j-1
                    if j + 1 < NJ:
                        for m in range(4):
                            filler.extend(qk_chunk_steps(m, j + 1))
                        for t in range(4 * (j + 1), 4 * (j + 1) + 4):
                            filler.extend(v_tile_steps(t))
                    if j >= 1:
                        filler.extend(outproj_steps(j - 1))
                    attn_block(0, j)
                    pump(4)
                    attn_block(1, j)
                    pump(4)
                drain()
                for f in outproj_steps(NJ - 1):
                    f()
    nc.compile()
    return nc


def _get_runner(reps=1):
    """Compile once; return a callable(in_maps) -> list of per-core out dicts."""
    key = ("runner", reps)
    if key in _CACHE:
        return _CACHE[key]
    import jax
    from jax.sharding import Mesh, PartitionSpec
    from jax.experimental.shard_map import shard_map
    from concourse import bass2jax

    nc = build_nc(reps)
    bass2jax.install_neuronx_cc_hook()

    partition_name = (nc.partition_id_tensor.name
                      if nc.partition_id_tensor else None)
    in_names, out_names, out_avals, zero_outs = [], [], [], []
    for alloc in nc.m.functions[0].allocations:
        if not isinstance(alloc, mybir.MemoryLocationSet):
            continue
        name = alloc.memorylocations[0].name
        if alloc.kind == "ExternalInput":
            if name != partition_name:
                in_names.append(name)
        elif alloc.kind == "ExternalOutput":
            out_names.append(name)
            shape = tuple(alloc.tensor_shape)
            dtype = mybir.dt.np(alloc.dtype)
            out_avals.append(jax.core.ShapedArray(shape, dtype))
            zero_outs.append(np.zeros(shape, dtype))
    n_params = len(in_names)
    n_outs = len(out_avals)
    all_in_names = list(in_names) + list(out_names)
    if partition_name is not None:
        all_in_names.append(partition_name)
    donate = tuple(range(n_params, n_params + n_outs))

    def _body(*args):
        operands = list(args)
        if partition_name is not None:
            operands.append(bass2jax.partition_id_tensor())
        outs = bass2jax._bass_exec_p.bind(
            *operands,
            out_avals=tuple(out_avals),
            in_names=tuple(all_in_names),
            out_names=tuple(out_names),
            lowering_input_output_aliases=(),
            sim_require_finite=True,
            sim_require_nnan=True,
            nc=nc,
        )
        return tuple(outs)

    n_cores = 8
    devices = jax.devices()[:n_cores]
    mesh = Mesh(np.asarray(devices), ("core",))
    in_specs = (PartitionSpec("core"),) * (n_params + n_outs)
    out_specs = (PartitionSpec("core"),) * n_outs
    sharded = jax.jit(
        shard_map(_body, mesh=mesh, in_specs=in_specs, out_specs=out_specs,
                  check_rep=False),
        donate_argnums=donate, keep_unused=True)

    def run(in_maps):
        per_core = [[np.asarray(m[name]) for name in in_names] for m in in_maps]
        concat_in = [np.concatenate([per_core[c][i] for c in range(n_cores)],
                                    axis=0) for i in range(n_params)]
        concat_zeros = [np.zeros((n_cores * z.shape[0], *z.shape[1:]), z.dtype)
                        for z in zero_outs]
        out_arrs = sharded(*concat_in, *concat_zeros)
        return [
            {name: np.asarray(out_arrs[i]).reshape(n_cores,
                                                   *out_avals[i].shape)[c]
             for i, name in enumerate(out_names)}
            for c in range(n_cores)
        ]

    _CACHE[key] = run
    return run


def _get_bench(reps=1, skip=()):
    """Zero-transfer bench callable: inputs pre-placed on device, outputs
    left on device (block_until_ready only). No donation."""
    key = ("bench", reps, tuple(skip))
    if key in _CACHE:
        return _CACHE[key]
    import jax
    from jax.sharding import Mesh, PartitionSpec, NamedSharding
    from jax.experimental.shard_map import shard_map
    from concourse import bass2jax

    nc = build_nc(reps, skip)
    bass2jax.install_neuronx_cc_hook()
    partition_name = (nc.partition_id_tensor.name
                      if nc.partition_id_tensor else None)
    in_names, out_names, out_avals, zero_outs = [], [], [], []
    for alloc in nc.m.functions[0].allocations:
        if not isinstance(alloc, mybir.MemoryLocationSet):
            continue
        name = alloc.memorylocations[0].name
        if alloc.kind == "ExternalInput":
            if name != partition_name:
                in_names.append(name)
        elif alloc.kind == "ExternalOutput":
            out_names.append(name)
            shape = tuple(alloc.tensor_shape)
            dtype = mybir.dt.np(alloc.dtype)
            out_avals.append(jax.core.ShapedArray(shape, dtype))
            zero_outs.append(np.zeros(shape, dtype))
    n_params = len(in_names)
    all_in_names = list(in_names) + list(out_names)
    if partition_name is not None:
        all_in_names.append(partition_name)

    def _body(*args):
        operands = list(args)
        if partition_name is not None:
            operands.append(bass2jax.partition_id_tensor())
        outs = bass2jax._bass_exec_p.bind(
            *operands,
            out_avals=tuple(out_avals),
            in_names=tuple(all_in_names),
            out_names=tuple(out_names),
            lowering_input_output_aliases=(),
            sim_require_finite=True,
            sim_require_nnan=True,
            nc=nc,
        )
        return tuple(outs)

    n_cores = 8
    devices = jax.devices()[:n_cores]
    mesh = Mesh(np.asarray(devices), ("core",))
    nouts = len(out_names)
    in_specs = (PartitionSpec("core"),) * (n_params + nouts)
    out_specs = (PartitionSpec("core"),) * nouts
    sharded = jax.jit(
        shard_map(_body, mesh=mesh, in_specs=in_specs, out_specs=out_specs,
                  check_rep=False),
        keep_unused=True)
    shard = NamedSharding(mesh, PartitionSpec("core"))

    def make_args(in_maps):
        per_core = [[np.asarray(m[name]) for name in in_names]
                    for m in in_maps]
        concat_in = [np.concatenate([per_core[c][i] for c in range(n_cores)],
                                    axis=0) for i in range(n_params)]
        concat_zeros = [np.zeros((n_cores * z.shape[0], *z.shape[1:]),
                                 z.dtype) for z in zero_outs]
        return [jax.device_put(a, shard) for a in concat_in + concat_zeros]

    def call(dev_args):
        outs = sharded(*dev_args)
        for o in outs:
            o.block_until_ready()
        return outs

    call.sharded = sharded
    result = (make_args, call)
    _CACHE[key] = result
    return result


def _prep_in_maps(x, w_qkv, w_out):
    import ml_dtypes
    bf16 = ml_dtypes.bfloat16
    x = np.asarray(x, dtype=np.float32)
    w_qkv = np.asarray(w_qkv, dtype=np.float32)
    w_out = np.asarray(w_out, dtype=np.float32)
    in_maps = []
    xts = [np.ascontiguousarray(x[b].T).astype(bf16) for b in range(B)]
    for core in range(8):
        b, g = divmod(core, 4)
        cl, ch = 256 * g, 256 * g + 256
        wqk = np.ascontiguousarray(
            np.concatenate([w_qkv[:, cl:ch], w_qkv[:, C + cl:C + ch]],
                           axis=1)).astype(bf16)
        wv = np.ascontiguousarray(w_qkv[:, 2 * C + cl:2 * C + ch]).astype(bf16)
        wo = np.ascontiguousarray(w_out[cl:ch, :]).astype(np.float32)
        in_maps.append({"xt": xts[b], "wqk": wqk, "wv": wv, "wo": wo,
                        "ones_c": np.ones((128, 64), dtype=np.float32),
                        "ones_b": np.ones((128, 64), dtype=bf16)})
    return in_maps


def kernel(x, w_qkv, w_out):
    run = _get_runner()
    in_maps = _prep_in_maps(x, w_qkv, w_out)
    results = run(in_maps)
    y = np.zeros((B, T, C), dtype=np.float32)
    for core in range(8):
        b = core // 4
        y[b] += results[core]["y"]
    return y


if __name__ == "__main__":
    rng = np.random.default_rng(0)
    x = rng.standard_normal((B, T, C)).astype(np.float32)
    w_qkv = (rng.standard_normal((C, 3 * C)) / np.sqrt(C)).astype(np.float32)
    w_out = (rng.standard_normal((C, C)) / np.sqrt(C)).astype(np.float32)
    y = kernel(x=x, w_qkv=w_qkv, w_out=w_out)
    print("kernel ran, y:", y.shape, y.dtype, float(np.abs(y).max()))


# revision 18
# speedup vs baseline: 1.2871x; 1.1554x over previous
"""Trainium2 Bass kernel for multi-head causal attention (v2).

Problem (hardcoded): x [2, 2048, 1024] fp32, w_qkv [1024, 3072], w_out [1024, 1024].
  qkv = x @ w_qkv; per-head causal softmax attention (16 heads, d=64);
  out = attn_out @ w_out.

Sharding: 8 cores = (2 batches) x (4 head-groups of 4 heads).
v2 changes vs v1:
  - all inputs bf16 (halves DMA + SBUF, FWL on weight loads)
  - S^T matmuls row-paired across the two heads of a pair (concurrent
    64-row tile_position groups)
  - software-pipelined emission: St(g+1) ahead of Sv(g); a filler queue
    interleaves projection chunks + out-proj of the previous q-chunk into
    the attention blocks to plug PE stalls
  - per-(pair, q-chunk) normalization (reciprocal + DMA broadcast) feeding
    a pipelined out-projection, instead of a serial phase-3/4 tail
  - narrowed causal affine_selects (only the invalid diagonal region)
"""
import numpy as np

import concourse.bass as bass
from concourse import bacc
import concourse.mybir as mybir
import concourse.tile as tile

F32 = mybir.dt.float32
F32R = mybir.dt.float32r
BF16 = mybir.dt.bfloat16
AF = mybir.ActivationFunctionType

B, T, C = 2, 2048, 1024
H_TOT, D = 16, 64
HL = 4             # heads per core
DL = HL * D        # 256 local channels
NJ = 4             # q-chunks of 512
NKT = 16           # k-tiles of 128
NCT = 8            # c-tiles of 128 (contraction over C)
SM_SCALE = 1.0 / np.sqrt(D)

_CACHE = {}


def build_nc(reps=1, skip=()):
    nc = bacc.Bacc("TRN2", target_bir_lowering=False)
    xt = nc.dram_tensor("xt", [C, T], BF16, kind="ExternalInput")
    wqk = nc.dram_tensor("wqk", [C, 2 * DL], BF16, kind="ExternalInput")
    wv = nc.dram_tensor("wv", [C, DL], BF16, kind="ExternalInput")
    wo = nc.dram_tensor("wo", [DL, C], BF16, kind="ExternalInput")
    ones_c = nc.dram_tensor("ones_c", [128, 64], BF16, kind="ExternalInput")
    ones_b = nc.dram_tensor("ones_b", [128, 64], BF16, kind="ExternalInput")
    y = nc.dram_tensor("y", [T, C], F32, kind="ExternalOutput")

    with tile.TileContext(nc) as tc:
      for _rep in range(reps):
        with tc.tile_pool(name="persist", bufs=1) as persist, \
             tc.tile_pool(name="dram", bufs=2, space="DRAM") as drampool:
            # ---- persistent SBUF tiles ----
            qk_tiles = [persist.tile([128, T], BF16, tag=f"qk{m}", name=f"qk{m}")
                        for m in range(4)]
            # qk layout: m=0 Q^T pair0, m=1 Q^T pair1, m=2 K^T pair0, m=3 K^T pair1
            v_sb = [persist.tile([128, HL, D + 1], BF16, tag=f"v{t}", name=f"v{t}")
                    for t in range(NKT)]
            at_t = [persist.tile([128, T], BF16, tag=f"at{p}", name=f"at{p}")
                    for p in range(2)]
            ones128 = persist.tile([128, 64], BF16, tag="ones128",
                                   name="ones128")
            xt_sb = [persist.tile([128, T], BF16, tag=f"xt{c}", name=f"xt{c}")
                     for c in range(NCT)]
            wqk_sb = [persist.tile([128, 2 * DL], BF16, tag=f"wqk{c}",
                                   name=f"wqk{c}") for c in range(NCT)]
            wv_sb = [persist.tile([128, DL], BF16, tag=f"wv{c}", name=f"wv{c}")
                     for c in range(NCT)]
            wo_sb = [persist.tile([128, C], BF16, tag=f"wo{i}", name=f"wo{i}")
                     for i in range(2)]

            with tc.tile_pool(name="pexp", bufs=3) as pexp, \
                 tc.tile_pool(name="norm", bufs=2) as normp, \
                 tc.tile_pool(name="ysb", bufs=4) as ysbp, \
                 tc.tile_pool(name="ps2", bufs=2, space="PSUM") as ps2, \
                 tc.tile_pool(name="pot", bufs=1, space="PSUM") as pot, \
                 tc.tile_pool(name="pwork", bufs=1, space="PSUM") as pwork:

                # ---- input DMA: column-chunked xt so chunk-0 work starts early
                nc.sync.dma_start(out=ones128[:], in_=ones_b[:, :])
                for jc in range(NJ):
                    for c in range(NCT):
                        eng = nc.sync if c % 2 == 0 else nc.scalar
                        eng.dma_start(
                            out=xt_sb[c][:, 512 * jc:512 * (jc + 1)],
                            in_=xt[128 * c:128 * (c + 1),
                                   512 * jc:512 * (jc + 1)])
                    if jc == 0:
                        for c in range(NCT):
                            nc.sync.dma_start(
                                out=wqk_sb[c][:],
                                in_=wqk[128 * c:128 * (c + 1), :])
                        for c in range(NCT):
                            nc.scalar.dma_start(
                                out=wv_sb[c][:],
                                in_=wv[128 * c:128 * (c + 1), :])
                        for i in range(2):
                            nc.scalar.dma_start(
                                out=wo_sb[i][:],
                                in_=wo[128 * i:128 * (i + 1), :])
                # ones column for the rowsum trick
                for t in range(NKT):
                    nc.scalar.dma_start(
                        out=v_sb[t][:, :, D:D + 1],
                        in_=ones_c[:, 0:HL].rearrange("p (h o) -> p h o", o=1))
                if "exp" in skip:
                    p2c = persist.tile([128, 1024], BF16, tag="p2c",
                                       name="p2c")
                    nc.vector.memset(p2c[:], 0.01)
                if "proj" in skip:
                    for m in range(4):
                        nc.vector.memset(qk_tiles[m][:], 0.01)
                    for t in range(NKT):
                        nc.vector.memset(v_sb[t][:, :, 0:D], 0.01)

                # ---- filler queue: small PE/DVE work units interleaved into
                # the attention blocks to plug engine stalls.
                filler = []

                def pump(k):
                    for _ in range(k):
                        if filler:
                            filler.pop(0)()

                def drain():
                    while filler:
                        filler.pop(0)()

                def qk_chunk_steps(m, j):
                    """Emit Q/K projection chunk as filler steps (2 MMs each)."""
                    if "proj" in skip:
                        return []
                    acc = {}

                    def mk(c0):
                        def f():
                            if c0 == 0:
                                acc["t"] = pwork.tile([128, 512], F32,
                                                      tag="acc", name="acc")
                            for c in (c0, c0 + 1):
                                nc.tensor.matmul(
                                    acc["t"][:],
                                    wqk_sb[c][:, 128 * m:128 * (m + 1)],
                                    xt_sb[c][:, 512 * j:512 * (j + 1)],
                                    start=(c == 0), stop=(c == NCT - 1))
                            if c0 == NCT - 2:
                                nc.vector.tensor_copy(
                                    qk_tiles[m][:, 512 * j:512 * (j + 1)],
                                    acc["t"][:])
                        return f
                    return [mk(c0) for c0 in range(0, NCT, 2)]

                def v_tile_steps(t):
                    if "proj" in skip:
                        return []
                    acc = {}

                    def mk(c0):
                        def f():
                            if c0 == 0:
                                acc["t"] = pwork.tile([128, 512], F32,
                                                      tag="acc", name="acc")
                            for c in (c0, c0 + 1):
                                nc.tensor.matmul(
                                    acc["t"][:, 0:DL],
                                    xt_sb[c][:, 128 * t:128 * (t + 1)],
                                    wv_sb[c][:],
                                    start=(c == 0), stop=(c == NCT - 1))
                            if c0 == NCT - 2:
                                nc.vector.tensor_copy(
                                    v_sb[t][:, :, 0:D],
                                    acc["t"][:, 0:DL].rearrange(
                                        "p (h d) -> p h d", h=HL))
                        return f
                    return [mk(c0) for c0 in range(0, NCT, 2)]

                def outproj_steps(j):
                    """Out-projection + DMA for t-tiles of q-chunk j."""
                    if "outproj" in skip:
                        return []
                    steps = []
                    for t in range(4 * j, 4 * j + 4):
                        for oc in range(2):
                            def f(t=t, oc=oc):
                                yps = pwork.tile([128, 512], F32, tag="y",
                                                 name="y")
                                for i in range(2):
                                    nc.tensor.matmul(
                                        yps[:],
                                        at_t[i][:, 128 * t:128 * (t + 1)],
                                        wo_sb[i][:, 512 * oc:512 * (oc + 1)],
                                        start=(i == 0), stop=(i == 1))
                                ysb = ysbp.tile([128, 512], F32, tag="ysb",
                                                name="ysb")
                                nc.vector.tensor_copy(ysb[:], yps[:])
                                nc.sync.dma_start(
                                    out=y[128 * t:128 * (t + 1),
                                          512 * oc:512 * (oc + 1)],
                                    in_=ysb[:])
                            steps.append(f)
                    return steps

                # ---- attention block for (pair, j), St(g+1) emitted ahead of
                # Sv(g); exp on ACT; masks narrowed to the invalid region.
                def st_group(pair, j, g, s2_t):
                    for kk in range(2):        # kk inner, h2 paired adjacent
                        for h2 in range(2):
                            base = 64 * h2
                            kt = 2 * g + kk
                            nc.tensor.matmul(
                                s2_t[h2][:, 512 * kk:512 * (kk + 1)],
                                qk_tiles[2 + pair][base:base + 64,
                                                   128 * kt:128 * (kt + 1)],
                                qk_tiles[pair][base:base + 64,
                                               512 * j:512 * (j + 1)],
                                start=True, stop=True)

                def attn_block(pair, j):
                    nkt = 4 * (j + 1)
                    ktgs = 2 * (j + 1)
                    ot = [pot.tile([65, 512], F32, tag=f"ot{h2}",
                                   name=f"ot{h2}") for h2 in range(2)]
                    s2_cur = [ps2.tile([128, 1024], F32, tag="s2", name="s2")
                              for _ in range(2)]
                    st_group(pair, j, 0, s2_cur)
                    for g in range(ktgs):
                        if "exp" in skip:
                            p2 = [p2c, p2c]
                        else:
                            p2 = [None, None]
                            for h2 in range(2):
                                p2[h2] = pexp.tile([128, 1024], BF16,
                                                   tag="p2", name="p2")
                                nc.scalar.activation(
                                    p2[h2][:], s2_cur[h2][:], AF.Exp,
                                    scale=float(SM_SCALE))
                        pump(2)
                        if g + 1 < ktgs:
                            s2_nxt = [ps2.tile([128, 1024], F32, tag="s2",
                                               name="s2") for _ in range(2)]
                            st_group(pair, j, g + 1, s2_nxt)
                        else:
                            s2_nxt = None
                        if g >= 2 * j and "mask" not in skip \
                                and "exp" not in skip:
                            for h2 in range(2):
                                for kk in range(2):
                                    r = 2 * g + kk - 4 * j
                                    w = 128 * (r + 1)
                                    nc.gpsimd.affine_select(
                                        out=p2[h2][:, 512 * kk:512 * kk + w],
                                        in_=p2[h2][:, 512 * kk:512 * kk + w],
                                        compare_op=mybir.AluOpType.is_ge,
                                        fill=0.0, base=-128 * r,
                                        pattern=[[1, w]],
                                        channel_multiplier=-1)
                        for h2 in range(2):
                            h = 2 * pair + h2
                            for kk in range(2):
                                kt = 2 * g + kk
                                nc.tensor.matmul(
                                    ot[h2][:],
                                    v_sb[kt][:, h, 0:D + 1],
                                    p2[h2][:, 512 * kk:512 * (kk + 1)],
                                    start=(kt == 0), stop=(kt == nkt - 1))
                        pump(2)
                        s2_cur = s2_nxt

                    # ---- per-chunk normalization ----
                    if "norm" in skip:
                        for h2 in range(2):
                            nc.vector.tensor_copy(
                                at_t[pair][64 * h2:64 * h2 + 64,
                                           512 * j:512 * (j + 1)],
                                ot[h2][0:64, :])
                        return
                    rs = normp.tile([1, 1024], F32, tag="rs", name="rs")
                    rsc = normp.tile([1, 1024], F32, tag="rsc", name="rsc")
                    scr = normp.tile([1, 1024], F32, tag="scr", name="scr")
                    rcr = normp.tile([128, 512], BF16, tag="rcr", name="rcr")
                    for h2 in range(2):
                        nc.vector.tensor_copy(
                            rs[0:1, 512 * h2:512 * (h2 + 1)],
                            ot[h2][64:65, :])
                        nc.vector.tensor_copy(
                            at_t[pair][64 * h2:64 * h2 + 64,
                                       512 * j:512 * (j + 1)],
                            ot[h2][0:64, :])
                    nc.vector.reciprocal_approx_accurate(rsc[:], rs[:], scr[:])
                    # stage recip rows at partitions 32 / 64, then broadcast
                    # across partitions with two K=1 matmuls (no DMA latency).
                    nc.vector.tensor_copy(rcr[32:33, :], rsc[0:1, 0:512])
                    nc.vector.tensor_copy(rcr[64:65, :], rsc[0:1, 512:1024])

                    def finish_norm(pair=pair, j=j, rcr=rcr):
                        bc = pwork.tile([128, 512], F32, tag="y", name="bc")
                        for h2 in range(2):
                            pb = 32 + 32 * h2
                            nc.tensor.matmul(
                                bc[64 * h2:64 * h2 + 64, :],
                                ones128[pb:pb + 1, 0:64],
                                rcr[pb:pb + 1, :],
                                start=True, stop=True)
                        bcs = normp.tile([128, 512], BF16, tag="bcs",
                                         name="bcs")
                        nc.vector.tensor_copy(bcs[:], bc[:])
                        nc.vector.tensor_mul(
                            at_t[pair][:, 512 * j:512 * (j + 1)],
                            at_t[pair][:, 512 * j:512 * (j + 1)], bcs[:])
                    filler.append(finish_norm)

                # ---- main schedule ----
                for m in range(4):
                    filler.extend(qk_chunk_steps(m, 0))
                for t in range(4):
                    filler.extend(v_tile_steps(t))
                drain()
                for j in range(NJ):
                    # stage filler: projections for j+1, out-proj for j-1
                    if j + 1 < NJ:
                        for m in range(4):
                            filler.extend(qk_chunk_steps(m, j + 1))
                        for t in range(4 * (j + 1), 4 * (j + 1) + 4):
                            filler.extend(v_tile_steps(t))
                    if j >= 1:
                        filler.extend(outproj_steps(j - 1))
                    attn_block(0, j)
                    pump(4)
                    attn_block(1, j)
                    pump(4)
                drain()
                for f in outproj_steps(NJ - 1):
                    f()
    nc.compile()
    return nc


def _get_runner(reps=1):
    """Compile once; return a callable(in_maps) -> list of per-core out dicts."""
    key = ("runner", reps)
    if key in _CACHE:
        return _CACHE[key]
    import jax
    from jax.sharding import Mesh, PartitionSpec
    from jax.experimental.shard_map import shard_map
    from concourse import bass2jax

    nc = build_nc(reps)
    bass2jax.install_neuronx_cc_hook()

    partition_name = (nc.partition_id_tensor.name
                      if nc.partition_id_tensor else None)
    in_names, out_names, out_avals, zero_outs = [], [], [], []
    for alloc in nc.m.functions[0].allocations:
        if not isinstance(alloc, mybir.MemoryLocationSet):
            continue
        name = alloc.memorylocations[0].name
        if alloc.kind == "ExternalInput":
            if name != partition_name:
                in_names.append(name)
        elif alloc.kind == "ExternalOutput":
            out_names.append(name)
            shape = tuple(alloc.tensor_shape)
            dtype = mybir.dt.np(alloc.dtype)
            out_avals.append(jax.core.ShapedArray(shape, dtype))
            zero_outs.append(np.zeros(shape, dtype))
    n_params = len(in_names)
    n_outs = len(out_avals)
    all_in_names = list(in_names) + list(out_names)
    if partition_name is not None:
        all_in_names.append(partition_name)
    donate = tuple(range(n_params, n_params + n_outs))

    def _body(*args):
        operands = list(args)
        if partition_name is not None:
            operands.append(bass2jax.partition_id_tensor())
        outs = bass2jax._bass_exec_p.bind(
            *operands,
            out_avals=tuple(out_avals),
            in_names=tuple(all_in_names),
            out_names=tuple(out_names),
            lowering_input_output_aliases=(),
            sim_require_finite=True,
            sim_require_nnan=True,
            nc=nc,
        )
        return tuple(outs)

    n_cores = 8
    devices = jax.devices()[:n_cores]
    mesh = Mesh(np.asarray(devices), ("core",))
    in_specs = (PartitionSpec("core"),) * (n_params + n_outs)
    out_specs = (PartitionSpec("core"),) * n_outs
    sharded = jax.jit(
        shard_map(_body, mesh=mesh, in_specs=in_specs, out_specs=out_specs,
                  check_rep=False),
        donate_argnums=donate, keep_unused=True)

    def run(in_maps):
        per_core = [[np.asarray(m[name]) for name in in_names] for m in in_maps]
        concat_in = [np.concatenate([per_core[c][i] for c in range(n_cores)],
                                    axis=0) for i in range(n_params)]
        concat_zeros = [np.zeros((n_cores * z.shape[0], *z.shape[1:]), z.dtype)
                        for z in zero_outs]
        out_arrs = sharded(*concat_in, *concat_zeros)
        return [
            {name: np.asarray(out_arrs[i]).reshape(n_cores,
                                                   *out_avals[i].shape)[c]
             for i, name in enumerate(out_names)}
            for c in range(n_cores)
        ]

    _CACHE[key] = run
    return run


def _get_bench(reps=1, skip=()):
    """Zero-transfer bench callable: inputs pre-placed on device, outputs
    left on device (block_until_ready only). No donation."""
    key = ("bench", reps, tuple(skip))
    if key in _CACHE:
        return _CACHE[key]
    import jax
    from jax.sharding import Mesh, PartitionSpec, NamedSharding
    from jax.experimental.shard_map import shard_map
    from concourse import bass2jax

    nc = build_nc(reps, skip)
    bass2jax.install_neuronx_cc_hook()
    partition_name = (nc.partition_id_tensor.name
                      if nc.partition_id_tensor else None)
    in_names, out_names, out_avals, zero_outs = [], [], [], []
    for alloc in nc.m.functions[0].allocations:
        if not isinstance(alloc, mybir.MemoryLocationSet):
            continue
        name = alloc.memorylocations[0].name
        if alloc.kind == "ExternalInput":
            if name != partition_name:
                in_names.append(name)
        elif alloc.kind == "ExternalOutput":
            out_names.append(name)
            shape = tuple(alloc.tensor_shape)
            dtype = mybir.dt.np(alloc.dtype)
            out_avals.append(jax.core.ShapedArray(shape, dtype))
            zero_outs.append(np.zeros(shape, dtype))
    n_params = len(in_names)
    all_in_names = list(in_names) + list(out_names)
    if partition_name is not None:
        all_in_names.append(partition_name)

    def _body(*args):
        operands = list(args)
        if partition_name is not None:
            operands.append(bass2jax.partition_id_tensor())
        outs = bass2jax._bass_exec_p.bind(
            *operands,
            out_avals=tuple(out_avals),
            in_names=tuple(all_in_names),
            out_names=tuple(out_names),
            lowering_input_output_aliases=(),
            sim_require_finite=True,
            sim_require_nnan=True,
            nc=nc,
        )
        return tuple(outs)

    n_cores = 8
    devices = jax.devices()[:n_cores]
    mesh = Mesh(np.asarray(devices), ("core",))
    nouts = len(out_names)
    in_specs = (PartitionSpec("core"),) * (n_params + nouts)
    out_specs = (PartitionSpec("core"),) * nouts
    sharded = jax.jit(
        shard_map(_body, mesh=mesh, in_specs=in_specs, out_specs=out_specs,
                  check_rep=False),
        keep_unused=True)
    shard = NamedSharding(mesh, PartitionSpec("core"))

    def make_args(in_maps):
        per_core = [[np.asarray(m[name]) for name in in_names]
                    for m in in_maps]
        concat_in = [np.concatenate([per_core[c][i] for c in range(n_cores)],
                                    axis=0) for i in range(n_params)]
        concat_zeros = [np.zeros((n_cores * z.shape[0], *z.shape[1:]),
                                 z.dtype) for z in zero_outs]
        return [jax.device_put(a, shard) for a in concat_in + concat_zeros]

    def call(dev_args):
        outs = sharded(*dev_args)
        for o in outs:
            o.block_until_ready()
        return outs

    call.sharded = sharded
    result = (make_args, call)
    _CACHE[key] = result
    return result


def _prep_in_maps(x, w_qkv, w_out):
    import ml_dtypes
    bf16 = ml_dtypes.bfloat16
    x = np.asarray(x, dtype=np.float32)
    w_qkv = np.asarray(w_qkv, dtype=np.float32)
    w_out = np.asarray(w_out, dtype=np.float32)
    in_maps = []
    xts = [np.ascontiguousarray(x[b].T).astype(bf16) for b in range(B)]
    for core in range(8):
        b, g = divmod(core, 4)
        cl, ch = 256 * g, 256 * g + 256
        wqk = np.ascontiguousarray(
            np.concatenate([w_qkv[:, cl:ch], w_qkv[:, C + cl:C + ch]],
                           axis=1)).astype(bf16)
        wv = np.ascontiguousarray(w_qkv[:, 2 * C + cl:2 * C + ch]).astype(bf16)
        wo = np.ascontiguousarray(w_out[cl:ch, :]).astype(bf16)
        in_maps.append({"xt": xts[b], "wqk": wqk, "wv": wv, "wo": wo,
                        "ones_c": np.ones((128, 64), dtype=bf16),
                        "ones_b": np.ones((128, 64), dtype=bf16)})
    return in_maps


def kernel(x, w_qkv, w_out):
    run = _get_runner()
    in_maps = _prep_in_maps(x, w_qkv, w_out)
    results = run(in_maps)
    y = np.zeros((B, T, C), dtype=np.float32)
    for core in range(8):
        b = core // 4
        y[b] += results[core]["y"]
    return y


if __name__ == "__main__":
    rng = np.random.default_rng(0)
    x = rng.standard_normal((B, T, C)).astype(np.float32)
    w_qkv = (rng.standard_normal((C, 3 * C)) / np.sqrt(C)).astype(np.float32)
    w_out = (rng.standard_normal((C, C)) / np.sqrt(C)).astype(np.float32)
    y = kernel(x=x, w_qkv=w_qkv, w_out=w_out)
    print("kernel ran, y:", y.shape, y.dtype, float(np.abs(y).max()))
